# revision 1
# baseline (speedup 1.0000x reference)
"""Single-head attention kernel for Trainium2, 8 NeuronCores.

Problem: x[4, 4096, 1024] f32; Wq/Wk/Wv [1024, 64]; bq/bk/bv [64].
  Q/K/V = x @ W + b ; out = softmax(Q K^T / 8) @ V  -> [4, 4096, 64]

Sharding: 8 shards = (batch b, query-half h). Each core computes K/V for
all 4096 tokens of its batch and attention for its 2048 queries.

Design (single fused pipeline; ScalarE exp is the ~70us floor, PE kept
HAM-warm at 2.4GHz and ACT kept saturated):
  - x arrives host-pretiled as xH[128, c, k, t]: each chunk DMA is 128
    descriptors x 8KB. No tiny-packet DMAs: the softmax ones-column is
    memset on device, biases ship packed as one [128,4] tensor.
  - Warmup: the exp ACT table is preloaded and a few matmuls on a zeroed
    tile run during the input DMA so the PE HAM clock-gate is already at
    K=8/8 when real work starts.
  - K^T is split: even chunks pack [Wk|Wv] (K rows on partitions 0:64),
    odd chunks pack [Wv|Wk] (K on 64:128). Scores use ROW-TILED matmul
    pairs: two K=64 matmuls on row groups (0,0)/(64,0) run concurrently
    -> ~2x S^T throughput. Q^T is duplicated on both partition halves
    for free via a [Wq|Wq] lhsT.
  - The schedule weaves projection sub-parts (4 matmuls) between early
    attention slots so ACT starts ~20us in and stays ~90% busy; AV
    matmuls are deferred into the post-projection phase and drained at a
    decaying-backlog pace so the PE always has dense back-to-back work.
    V^T->V PE-transposes also run there as filler.
  - PSUM: ramp scope = 2x2-bank score tiles + 2x1-bank projection
    accumulators; dense scope reuses those 6 banks as 2x3-bank score
    tiles (N=1536 exps); 1 bank for qp/AV-accumulator (tag-shared), 1
    for transposes.
  - Epilogue: the raw [65,512] AV accumulator (numerator rows 0:64,
    ones-matmul denominator row 64) is copied to SBUF and DMA'd out;
    the host divides and transposes.
"""

from contextlib import ExitStack

import ml_dtypes
import numpy as np

import concourse.bass as bass
import concourse.mybir as mybir
from concourse import bacc
import concourse.tile as tile
from concourse.bass_utils import run_bass_kernel_spmd

B = 4
S = 4096
D = 1024
H = 64
NCORES = 8
TQ = S // 2      # queries per core
CH = 512         # token chunk for projections
QS = 512         # query slice for attention
NKT = D // 128   # 8 contraction tiles for projections
NCH = S // CH    # 8 token chunks
NK2 = S // 128   # 32 key tiles for attention
NQS = TQ // QS   # 4 query slices
NPAIR = NK2 // 2  # 16 row-tiled score pairs per query slice
SCALE = 1.0 / 8.0  # 1/sqrt(64)

F32 = mybir.dt.float32
F32R = mybir.dt.float32r
BF16 = mybir.dt.bfloat16


def k2_of_slot(half, p):
    """Global key-tile index for pair p's lo/hi slot.

    lo slot p comes from even chunk 2*(p//4), tile p%4 within it;
    hi slot p from odd chunk 2*(p//4)+1.
    """
    g, i = divmod(p, 4)
    return 8 * g + i + (4 if half else 0)


def build_nc():
    nc = bacc.Bacc(None, target_bir_lowering=False)
    xH = nc.dram_tensor("xH", [128, NCH, NKT, CH], BF16, kind="ExternalInput")
    wkv_e = nc.dram_tensor("wkv_e", [128, NKT * 128], BF16, kind="ExternalInput")
    wkv_o = nc.dram_tensor("wkv_o", [128, NKT * 128], BF16, kind="ExternalInput")
    wq2 = nc.dram_tensor("wq2", [128, NKT * 128], BF16, kind="ExternalInput")
    # biasd cols: 0=[bk;bv], 1=[bv;bk], 2=[bq;bq]
    biasd = nc.dram_tensor("biasd", [128, 4], F32, kind="ExternalInput")
    identd = nc.dram_tensor("identd", [128, 128], F32R, kind="ExternalInput")
    # raw AV accumulator per qs: rows 0:64 = O^T numerator, row 64 = softmax
    # denominator; the host divides and transposes.
    out = nc.dram_tensor("out", [NQS, 65, QS], F32, kind="ExternalOutput")

    with ExitStack() as ctx:
        tc = ctx.enter_context(tile.TileContext(nc))
        singles = ctx.enter_context(tc.tile_pool(name="singles", bufs=1))
        persist = ctx.enter_context(tc.tile_pool(name="persist", bufs=1))

        # K^T split by partition half: [0:64] = lo slots, [64:128] = hi.
        KT = persist.tile([128, NPAIR * 128], BF16)
        QT2 = persist.tile([128, TQ], BF16)     # Q^T duplicated on both halves
        Vaug = persist.tile([128, NK2, 65], BF16)  # V natural + ones col

        with (
            tc.tile_pool(name="xt", bufs=3) as xt_pool,
            tc.tile_pool(name="vt", bufs=8) as vt_pool,
            tc.tile_pool(name="p", bufs=28) as p_pool,
            tc.tile_pool(name="p2", bufs=6) as p2_pool,
            tc.tile_pool(name="osb", bufs=2) as osb_pool,
            tc.tile_pool(name="oqps", bufs=1, space="PSUM") as oq_ps_pool,
            tc.tile_pool(name="tpps", bufs=1, space="PSUM") as tp_ps_pool,
        ):
            kvst = {}  # c -> [xtc, kvp, vt, (qp)] chunk state

            # ---- attention slot machinery (unit = one 512-wide half) ----
            # A half h of pair p is key-tile k2_of_slot(h, p); lo halves use
            # array row group 0, hi halves row group 64. AV drains strictly
            # qs-by-qs: with oq bufs=1 an interleaved drain would deadlock.
            slot_qs = {q: [] for q in range(NQS)}  # qs -> [(k2, view512)]
            av_ptr = [0]
            ops = {}
            av_done = {}  # qs -> halves issued
            n_half = [0]
            n_av = [0]

            def emit_stN(halves, pool, st_pool):
                # one ACT slot covering len(halves) 512-wide score columns
                w = len(halves) * QS
                st = st_pool.tile([128, w], F32, name="st")
                for i, (qs, half, p) in enumerate(halves):
                    rows = slice(64, 128) if half else slice(0, 64)
                    nc.tensor.matmul(
                        st[:, i * QS : (i + 1) * QS],
                        KT[rows, p * 128 : (p + 1) * 128],
                        QT2[rows, qs * QS : (qs + 1) * QS],
                        start=True,
                        stop=True,
                    )
                p_tile = pool.tile([128, w], BF16, name="pt")
                nc.scalar.activation(
                    p_tile, st, mybir.ActivationFunctionType.Exp, scale=SCALE
                )
                for i, (qs, half, p) in enumerate(halves):
                    slot_qs[qs].append(
                        (k2_of_slot(half, p), p_tile[:, i * QS : (i + 1) * QS])
                    )
                n_half[0] += len(halves)

            def emit_av():
                """Issue one AV matmul (one half) for the lowest unfinished
                qs. Returns False if it has no issued-but-undrained half."""
                qs = av_ptr[0]
                if qs >= NQS or not slot_qs[qs]:
                    return False
                k2, view = slot_qs[qs].pop(0)
                if qs not in ops:
                    ops[qs] = oq_ps_pool.tile([128, QS], F32, name="op", tag="oq")
                    av_done[qs] = 0
                n = av_done[qs]
                nc.tensor.matmul(
                    ops[qs][0:65, :],
                    Vaug[:, k2, 0:65],
                    view,
                    start=(n == 0),
                    stop=(n == NK2 - 1),
                )
                av_done[qs] += 1
                n_av[0] += 1
                if av_done[qs] == NK2:
                    # raw numerator+denominator to DRAM (via one SBUF hop);
                    # the host normalizes and transposes.
                    osb = osb_pool.tile([65, QS], F32, name="osb")
                    nc.vector.tensor_copy(osb, ops.pop(qs)[0:65, :])
                    nc.sync.dma_start(out[qs, :, :], osb)
                    av_ptr[0] += 1
                return True

            with (
                tc.tile_pool(name="stps", bufs=2, space="PSUM") as st1_pool,
                tc.tile_pool(name="kvps", bufs=2, space="PSUM") as kv_ps_pool,
            ):
                # Warmup: preload the exp ACT table and run matmuls with no
                # readers so the PE HAM un-throttles during the DMA head.
                wrm = singles.tile([128, QS], BF16)
                nc.vector.memset(wrm, 0.0)
                wrm2 = singles.tile([128, 32], BF16)
                nc.scalar.activation(wrm2, wrm[:, 0:32],
                                     mybir.ActivationFunctionType.Exp,
                                     scale=SCALE)
                for _ in range(16):
                    wps = st1_pool.tile([128, 2 * QS], F32, name="st")
                    nc.tensor.matmul(wps[:, 0:QS], wrm[:, 0:128], wrm,
                                     start=True, stop=True)
                nc.vector.memset(Vaug[:, :, 64:65], 1.0)

                wkv_e_sb = singles.tile([128, NKT * 128], BF16)
                nc.sync.dma_start(wkv_e_sb, wkv_e[:, :])
                bias_sb = singles.tile([128, 4], F32)
                nc.sync.dma_start(bias_sb, biasd[:, :])
                wq2_sb = singles.tile([128, NKT * 128], BF16)
                nc.sync.dma_start(wq2_sb, wq2[:, :])
                wkv_o_sb = singles.tile([128, NKT * 128], BF16)
                ident = singles.tile([128, 128], F32R)

                def kv_a(c):
                    xtc = xt_pool.tile([128, NKT, CH], BF16, name="xtc")
                    nc.sync.dma_start(xtc, xH[:, c, :, :])
                    if c == 0:
                        # late singles: needed only from kv1 / kv_c(0)
                        nc.sync.dma_start(wkv_o_sb, wkv_o[:, :])
                        nc.sync.dma_start(ident, identd[:, :])
                    kvp = kv_ps_pool.tile([128, CH], F32, name="kvp")
                    kvst[c] = [xtc, kvp, None]
                    wsel = wkv_e_sb if c % 2 == 0 else wkv_o_sb
                    for kt in range(4):
                        nc.tensor.matmul(
                            kvp,
                            wsel[:, kt * 128 : (kt + 1) * 128],
                            xtc[:, kt, :],
                            start=(kt == 0),
                            stop=False,
                        )

                def kv_b(c):
                    xtc, kvp, _ = kvst[c]
                    wsel = wkv_e_sb if c % 2 == 0 else wkv_o_sb
                    bsel = bias_sb[:, 0:1] if c % 2 == 0 else bias_sb[:, 1:2]
                    for kt in range(4, NKT):
                        nc.tensor.matmul(
                            kvp,
                            wsel[:, kt * 128 : (kt + 1) * 128],
                            xtc[:, kt, :],
                            start=False,
                            stop=(kt == NKT - 1),
                        )
                    krows = slice(0, 64) if c % 2 == 0 else slice(64, 128)
                    vrows = slice(64, 128) if c % 2 == 0 else slice(0, 64)
                    pslot = c // 2
                    nc.vector.tensor_scalar_add(
                        KT[krows, 4 * pslot * 128 : (4 * pslot + 4) * 128],
                        kvp[krows, :],
                        bsel[krows, :],
                    )
                    vt = vt_pool.tile([128, CH], F32R, name="vt")
                    nc.vector.tensor_scalar_add(
                        vt[vrows, :], kvp[vrows, :], bsel[vrows, :]
                    )
                    kvst[c][2] = vt

                def q_a(c):
                    xtc = kvst[c][0]
                    qp = oq_ps_pool.tile([128, CH], F32, name="qp", tag="oq")
                    kvst[c].append(qp)
                    for kt in range(4):
                        nc.tensor.matmul(
                            qp,
                            wq2_sb[:, kt * 128 : (kt + 1) * 128],
                            xtc[:, kt, :],
                            start=(kt == 0),
                            stop=False,
                        )

                def q_b(c):
                    xtc, _, _, qp = kvst[c]
                    for kt in range(4, NKT):
                        nc.tensor.matmul(
                            qp,
                            wq2_sb[:, kt * 128 : (kt + 1) * 128],
                            xtc[:, kt, :],
                            start=False,
                            stop=(kt == NKT - 1),
                        )
                    nc.vector.tensor_scalar_add(
                        QT2[:, c * CH : (c + 1) * CH], qp, bias_sb[:, 2:3]
                    )

                def emit_st(qs, p):
                    emit_stN([(qs, 0, p), (qs, 1, p)], p_pool, st1_pool)

                # ---- ramp: fused projections + early attention slots ----
                kv_a(0); kv_b(0); q_a(0); q_b(0); kv_a(1); kv_b(1)

                parts = (
                    [("qa", 1), ("qb", 1)]
                    + [x for c in (2, 3) for x in
                       [("kva", c), ("kvb", c), ("qa", c), ("qb", c)]]
                    + [x for c in (4, 5, 6, 7) for x in
                       [("kva", c), ("kvb", c)]]
                )
                part_fn = {"kva": kv_a, "kvb": kv_b, "qa": q_a, "qb": q_b}
                kvb_done = 1
                qb_done = [True, False, False, False]
                next_pair = [0, 0, 0, 0]

                def slots_avail(qs):
                    if qs >= 2 or (qs == 1 and not qb_done[1]):
                        return False
                    p = next_pair[qs]
                    return p < NPAIR and 2 * (p // 4) + 1 <= kvb_done

                def take_slot(qs):
                    p = next_pair[qs]
                    next_pair[qs] += 1
                    emit_st(qs, p)

                for i, (kind, c) in enumerate(parts):
                    want = 1 if i < len(parts) - 6 else 2
                    for qs in (0, 1):
                        if want and slots_avail(qs):
                            take_slot(qs)
                            want -= 1
                    if want and slots_avail(0):
                        take_slot(0)
                    part_fn[kind](c)
                    if kind == "kvb":
                        kvb_done = c
                    if kind == "qb":
                        qb_done[c] = True

            # ---- dense phase: N=1536 ACT slots; ramp PSUM pools are closed,
            # freeing 6 banks for 2-deep 3-bank score tiles. V transposes
            # (kv_c) run here as PE filler.
            with tc.tile_pool(name="st2ps", bufs=2, space="PSUM") as st2_pool:

                def kv_c(c):
                    vrows = slice(64, 128) if c % 2 == 0 else slice(0, 64)
                    vt = kvst[c][2]
                    for s4 in range(CH // 128):
                        t2 = tp_ps_pool.tile([128, 128], F32, name="t2",
                                             tag="tp")
                        nc.tensor.transpose(
                            t2[:, 0:64].bitcast(F32R),
                            vt[vrows, s4 * 128 : (s4 + 1) * 128],
                            ident[vrows, vrows],
                        )
                        nc.vector.tensor_copy(
                            Vaug[:, c * (CH // 128) + s4, 0:64], t2[:, 0:64]
                        )

                dense = []
                for qs in range(NQS):
                    start_p = next_pair[qs] if qs < 2 else 0
                    dense += [(qs, h, p) for p in range(start_p, NPAIR)
                              for h in (0, 1)]
                backlog0 = n_half[0] - n_av[0]
                n_dslot = (len(dense) + 2) // 3
                for j in range(n_dslot):
                    sl = dense[3 * j : 3 * j + 3]
                    emit_stN(sl, p2_pool if len(sl) == 3 else p_pool, st2_pool)
                    if j < 4:
                        kv_c(2 * j)
                        kv_c(2 * j + 1)
                    if j < 2:
                        floor = backlog0  # no AV drain: feed ACT across the
                        # pool transition before burning PE on the backlog
                    else:
                        floor = max(2, (backlog0 * (n_dslot - 1 - j)) // n_dslot)
                    while (n_half[0] - n_av[0]) > floor and emit_av():
                        pass
                while emit_av():
                    pass
                assert n_av[0] == NQS * NK2, n_av[0]
    return nc


_NC_CACHE = None


def _get_nc():
    global _NC_CACHE
    if _NC_CACHE is None:
        nc = build_nc()
        nc.finalize()
        _NC_CACHE = nc
    return _NC_CACHE


LAST_RESULT = None
RUN_KWARGS = {}


def kernel(x, Wq, bq, Wk, bk, Wv, bv):
    global LAST_RESULT
    x = np.asarray(x, dtype=np.float32)
    Wq = np.asarray(Wq, dtype=np.float32)
    Wk = np.asarray(Wk, dtype=np.float32)
    Wv = np.asarray(Wv, dtype=np.float32)
    bq_a = np.asarray(bq, dtype=np.float32)
    bk_a = np.asarray(bk, dtype=np.float32)
    bv_a = np.asarray(bv, dtype=np.float32)

    bf = ml_dtypes.bfloat16

    # per 128-row contraction tile [128, kt, 128]: even = [Wk|Wv], odd = [Wv|Wk]
    def pack2(wa, wb):
        h = np.empty((128, NKT, 128), np.float32)
        h[:, :, :64] = wa.reshape(NKT, 128, 64).transpose(1, 0, 2)
        h[:, :, 64:] = wb.reshape(NKT, 128, 64).transpose(1, 0, 2)
        return np.ascontiguousarray(h.reshape(128, NKT * 128)).astype(bf)

    wkv_e_host = pack2(Wk, Wv)
    wkv_o_host = pack2(Wv, Wk)
    wq2_host = pack2(Wq, Wq)
    bias_host = np.zeros((128, 4), np.float32)
    bias_host[:, 0] = np.concatenate([bk_a, bv_a])
    bias_host[:, 1] = np.concatenate([bv_a, bk_a])
    bias_host[:, 2] = np.concatenate([bq_a, bq_a])
    ident_host = np.eye(128, dtype=np.float32)

    in_maps = []
    for c in range(NCORES):
        b, h = divmod(c, 2)
        xb = x[b]
        if h == 1:
            xb = np.concatenate([xb[TQ:], xb[:TQ]], axis=0)
        # xH[p, c, k, t] = x^T[k*128+p, c*512+t]
        xh = np.ascontiguousarray(
            xb.T.astype(bf).reshape(NKT, 128, NCH, CH).transpose(1, 2, 0, 3)
        ).reshape(128, NCH, NKT, CH)
        in_maps.append(
            {
                "xH": xh,
                "wkv_e": wkv_e_host,
                "wkv_o": wkv_o_host,
                "wq2": wq2_host,
                "biasd": bias_host,
                "identd": ident_host,
            }
        )

    nc = _get_nc()
    res = run_bass_kernel_spmd(nc, in_maps, core_ids=list(range(NCORES)), **RUN_KWARGS)
    LAST_RESULT = res

    outp = np.empty((B, S, H), np.float32)
    for c in range(NCORES):
        b, h = divmod(c, 2)
        o = res.results[c]["out"]  # [qs, 65, 512] raw numerator/denominator
        num = o[:, 0:64, :]                      # [qs, h, q]
        den = o[:, 64:65, :]
        outp[b, h * TQ : (h + 1) * TQ] = (
            (num / den).transpose(0, 2, 1).reshape(TQ, H)
        )
    return outp

